# revision 14
# baseline (speedup 1.0000x reference)
"""PointNet++ FP module (3-NN interpolate + SharedMLP) on 8 Trainium2 cores.

Sharding: B=4 batches x 2 halves of N=8192 -> 8 cores, 4096 points each.
Per core: -d2 via split-fp16 matmuls (hi*hi K=5 + cross K=10, exact to ~5e-6)
-> ACT fp32 copy -> DVE top-8 (max/max_index, exact selection) -> per-slot
indirect-DMA feature row gathers -> weighted sum + transpose fused into PE
matmuls against diag(w_k) -> fp16 MLP with folded BN + ReLU.
"""

import os
import numpy as np

import concourse.bass as bass
import concourse.tile as tile
from concourse import bacc, mybir
from concourse.bass import IndirectOffsetOnAxis
from concourse.bass_utils import run_bass_kernel_spmd

F32 = mybir.dt.float32
F16 = mybir.dt.float16
U32 = mybir.dt.uint32
AX = mybir.AxisListType
OP = mybir.AluOpType
ACTF = mybir.ActivationFunctionType

B, N, M = 4, 8192, 2048
C1, C2 = 128, 256
P = N // 2          # points per core
NT = P // 128       # 32 n-tiles per core
NG = NT // 4        # 8 groups of 4 tiles (512 points)
BN_EPS = 1e-5

_cache = {}


def _build(dbg=False):
    nc = bacc.Bacc("TRN2", target_bir_lowering=False, debug=False, num_devices=8)

    d_uh = nc.dram_tensor("uh", [5, P], F16, kind="ExternalInput").ap()
    d_ux = nc.dram_tensor("ux", [10, P], F16, kind="ExternalInput").ap()
    d_kh = nc.dram_tensor("kh", [5, M], F16, kind="ExternalInput").ap()
    d_kx = nc.dram_tensor("kx", [10, M], F16, kind="ExternalInput").ap()
    d_feats = nc.dram_tensor("feats16", [M, C2], F16, kind="ExternalInput").ap()
    d_ufeats = nc.dram_tensor("ufeats16", [C1, P], F16, kind="ExternalInput").ap()
    d_w0 = [nc.dram_tensor(f"w0{i}", [128, 256], F16, kind="ExternalInput").ap() for i in range(3)]
    d_w1 = [nc.dram_tensor(f"w1{i}", [128, 256], F16, kind="ExternalInput").ap() for i in range(2)]
    d_b0 = nc.dram_tensor("b0sb", [128, 2], F32, kind="ExternalInput").ap()
    d_b1 = nc.dram_tensor("b1sb", [128, 2], F32, kind="ExternalInput").ap()
    d_ident = nc.dram_tensor("ident", [128, 128], F16, kind="ExternalInput").ap()
    d_epsb = nc.dram_tensor("epsb", [128, 1], F32, kind="ExternalInput").ap()
    d_out = nc.dram_tensor("out", [C2, P], F32, kind="ExternalOutput").ap()
    d_dbg = {}
    if dbg:
        for nm, shp, dt in [
            ("dbg_top8", [128, 8], F32), ("dbg_idx8", [128, 32], U32),
            ("dbg_wt", [128, 12], F32), ("dbg_gf", [128, 3072], F16),
            ("dbg_xa", [128, 512], F16), ("dbg_h1a", [128, 512], F16),
        ]:
            d_dbg[nm] = nc.dram_tensor(nm, shp, dt, kind="ExternalOutput").ap()

    with tile.TileContext(nc) as tc:
        with (
            tc.tile_pool(name="const", bufs=1) as cpool,
            tc.tile_pool(name="neg", bufs=3) as negpool,
            tc.tile_pool(name="sel", bufs=2) as selpool,
            tc.tile_pool(name="mlp", bufs=2) as mlppool,
            tc.tile_pool(name="psum_neg", bufs=2, space="PSUM") as pneg,
            tc.tile_pool(name="psum_mlp", bufs=4, space="PSUM") as pmlp,
        ):
            # ---- persistent constants ----
            uh = cpool.tile([5, P], F16)
            nc.sync.dma_start(uh[:], d_uh)
            ux = cpool.tile([10, P], F16)
            nc.sync.dma_start(ux[:], d_ux)
            kh = cpool.tile([5, M], F16)
            nc.sync.dma_start(kh[:], d_kh)
            kx = cpool.tile([10, M], F16)
            nc.sync.dma_start(kx[:], d_kx)
            w0 = []
            for i in range(3):
                w0t = cpool.tile([128, 256], F16, name=f"w0t{i}")
                nc.sync.dma_start(w0t[:], d_w0[i])
                w0.append(w0t)
            w1 = []
            for i in range(2):
                w1t = cpool.tile([128, 256], F16, name=f"w1t{i}")
                nc.sync.dma_start(w1t[:], d_w1[i])
                w1.append(w1t)
            b0 = cpool.tile([128, 2], F32)
            nc.sync.dma_start(b0[:], d_b0)
            b1 = cpool.tile([128, 2], F32)
            nc.sync.dma_start(b1[:], d_b1)
            ident = cpool.tile([128, 128], F16)
            nc.sync.dma_start(ident[:], d_ident)
            epsb = cpool.tile([128, 1], F32)
            nc.sync.dma_start(epsb[:], d_epsb)

            for g in range(NG):
                # ---- selection: 4 tiles of 128 points ----
                idx8 = selpool.tile([128, 4, 8], U32, name=f"idx8_{g}")
                top8 = selpool.tile([128, 4, 8], F32, name=f"top8_{g}", tag="top8")
                for t in range(4):
                    i = g * 4 + t
                    negsb = negpool.tile([128, M], F32, name=f"negsb_{i}", tag="negsb")
                    for h in range(2):
                        pn = pneg.tile([128, 1024], F32, name=f"pn_{i}_{h}", tag="pn")
                        for c in range(2):
                            sl = slice(h * 1024 + c * 512, h * 1024 + (c + 1) * 512)
                            nc.tensor.matmul(
                                pn[:, c * 512:(c + 1) * 512],
                                uh[:, i * 128:(i + 1) * 128], kh[:, sl],
                                start=True, stop=False,
                            )
                            nc.tensor.matmul(
                                pn[:, c * 512:(c + 1) * 512],
                                ux[:, i * 128:(i + 1) * 128], kx[:, sl],
                                start=False, stop=True,
                            )
                        nc.scalar.activation(
                            negsb[:, h * 1024:(h + 1) * 1024], pn[:], ACTF.Copy
                        )
                    nc.vector.max(top8[:, t, :], negsb[:])
                    nc.vector.max_index(idx8[:, t, :], top8[:, t, :], negsb[:])
                if dbg and g == 0:
                    nc.sync.dma_start(d_dbg["dbg_top8"], top8[:, 0, :])
                    nc.sync.dma_start(d_dbg["dbg_idx8"], idx8[:].rearrange("p t k -> p (t k)"))

                # ---- weights: w_k = (1/(d_k+1e-8)) / sum ----
                dist = selpool.tile([128, 4, 3], F32, name=f"dist_{g}", tag="dist")
                nc.scalar.activation(dist[:], top8[:, :, 0:3], ACTF.Sqrt, bias=epsb[:], scale=-1.0)
                nc.vector.tensor_scalar_add(dist[:], dist[:], 1e-8)
                rec = selpool.tile([128, 4, 3], F32, name=f"rec_{g}", tag="rec")
                nc.vector.reciprocal(rec[:], dist[:])
                rs = selpool.tile([128, 4, 1], F32, name=f"rs_{g}", tag="rs")
                nc.vector.tensor_reduce(rs[:], rec[:], axis=AX.X, op=OP.add)
                nc.vector.reciprocal(rs[:], rs[:])
                wt = selpool.tile([128, 4, 3], F32, name=f"wt_{g}", tag="wt")
                nc.vector.tensor_mul(wt[:], rec[:], rs[:].broadcast_to((128, 4, 3)))
                if dbg and g == 0:
                    nc.sync.dma_start(d_dbg["dbg_wt"], wt[:].rearrange("p t k -> p (t k)"))

                # ---- gather feature rows (single-offset per call) ----
                gf = selpool.tile([128, 12, C2], F16, name=f"gf_{g}", tag="gf")
                for t in range(4):
                    for k in range(3):
                        nc.gpsimd.indirect_dma_start(
                            gf[:, t * 3 + k, :], None, d_feats,
                            IndirectOffsetOnAxis(ap=idx8[:, t, k:k + 1], axis=0),
                        )
                if dbg and g == 0:
                    nc.sync.dma_start(d_dbg["dbg_gf"], gf[:].rearrange("p t c -> p (t c)"))

                # ---- weighted sum + transpose fused into PE matmuls ----
                xa = mlppool.tile([128, 512], F16, name=f"xa_{g}", tag="xa")
                xb = mlppool.tile([128, 512], F16, name=f"xb_{g}", tag="xb")
                dgs = []
                for t in range(4):
                    for k in range(3):
                        dg = mlppool.tile([128, 128], F16, name=f"dg_{g}_{t}_{k}", tag="dg", bufs=6)
                        nc.gpsimd.tensor_scalar_mul(dg[:], ident[:], wt[:, t, k:k + 1])
                        dgs.append(dg)
                for t in range(4):
                    for o, dst in ((0, xa), (1, xb)):
                        pmx = pmlp.tile([128, 128], F32, name=f"pmx_{g}_{t}_{o}", tag="mlp")
                        for k in range(3):
                            nc.tensor.matmul(
                                pmx[:], gf[:, t * 3 + k, o * 128:(o + 1) * 128],
                                dgs[t * 3 + k][:], start=(k == 0), stop=(k == 2))
                        if o == 0:
                            nc.vector.tensor_copy(dst[:, t * 128:(t + 1) * 128], pmx[:])
                        else:
                            nc.scalar.activation(dst[:, t * 128:(t + 1) * 128], pmx[:], ACTF.Copy)
                xc = mlppool.tile([128, 512], F16, name=f"xc_{g}", tag="xc")
                nc.sync.dma_start(xc[:], d_ufeats[:, g * 512:(g + 1) * 512])
                if dbg and g == 0:
                    nc.sync.dma_start(d_dbg["dbg_xa"], xa[:])

                # ---- MLP ----
                xs = [xa, xb, xc]
                h1 = []
                for o in range(2):
                    pm = pmlp.tile([128, 512], F32, name=f"pm1_{g}_{o}", tag="mlp")
                    for kb in range(3):
                        nc.tensor.matmul(
                            pm[:], w0[kb][:, o * 128:(o + 1) * 128], xs[kb][:],
                            start=(kb == 0), stop=(kb == 2),
                        )
                    ho = mlppool.tile([128, 512], F16, name=f"h1_{g}_{o}", tag=f"h1{o}")
                    nc.scalar.activation(ho[:], pm[:], ACTF.Relu, bias=b0[:, o:o + 1])
                    h1.append(ho)
                if dbg and g == 0:
                    nc.sync.dma_start(d_dbg["dbg_h1a"], h1[0][:])
                for o in range(2):
                    pm2 = pmlp.tile([128, 512], F32, name=f"pm2_{g}_{o}", tag="mlp")
                    for kb in range(2):
                        nc.tensor.matmul(
                            pm2[:], w1[kb][:, o * 128:(o + 1) * 128], h1[kb][:],
                            start=(kb == 0), stop=(kb == 1),
                        )
                    ost = mlppool.tile([128, 512], F32, name=f"ost_{g}_{o}", tag=f"ost{o}")
                    nc.vector.tensor_scalar(
                        ost[:], pm2[:], b1[:, o:o + 1], 0.0, op0=OP.add, op1=OP.max
                    )
                    nc.sync.dma_start(d_out[o * 128:(o + 1) * 128, g * 512:(g + 1) * 512], ost[:])

    nc.compile()
    return nc


def _split16(a):
    hi = a.astype(np.float16)
    lo = (a - hi.astype(np.float32)).astype(np.float16)
    return hi, lo


def _prep_shared(inputs):
    sh = {}
    for li, (cin, cout) in enumerate([(C1 + C2, 256), (256, 256)]):
        W = np.asarray(inputs[f"W{li}"], np.float32)
        bb = np.asarray(inputs[f"b{li}"], np.float32)
        gg = np.asarray(inputs[f"g{li}"], np.float32)
        beta = np.asarray(inputs[f"beta{li}"], np.float32)
        rm = np.asarray(inputs[f"rm{li}"], np.float32)
        rv = np.asarray(inputs[f"rv{li}"], np.float32)
        scale = gg / np.sqrt(rv + BN_EPS)
        Wf = (W * scale[:, None]).astype(np.float16)
        bf = ((bb - rm) * scale + beta).astype(np.float32)
        wT = np.ascontiguousarray(Wf.T)                      # (cin, cout)
        for i in range(cin // 128):
            sh[f"w{li}{i}"] = np.ascontiguousarray(wT[i * 128:(i + 1) * 128])
        sh[f"b{li}sb"] = np.ascontiguousarray(bf.reshape(2, 128).T)
    sh["ident"] = np.eye(128, dtype=np.float16)
    sh["epsb"] = np.full((128, 1), 1e-16, np.float32)
    return sh


def _prep_core(inputs, shared, b, h):
    u = np.asarray(inputs["unknown"], np.float32)[b, h * P:(h + 1) * P]   # (P,3)
    k = np.asarray(inputs["known"], np.float32)[b]                        # (M,3)
    un2 = (u * u).sum(1)
    kn2 = (k * k).sum(1)
    uT5 = np.stack([2 * u[:, 0], 2 * u[:, 1], 2 * u[:, 2], un2, np.ones(P, np.float32)])
    kT5 = np.stack([k[:, 0], k[:, 1], k[:, 2], -np.ones(M, np.float32), -kn2])
    uh, ul = _split16(uT5)
    khh, kl = _split16(kT5)
    m = dict(shared)
    m["uh"] = np.ascontiguousarray(uh)
    m["kh"] = np.ascontiguousarray(khh)
    m["ux"] = np.ascontiguousarray(np.concatenate([uh, ul], 0))
    m["kx"] = np.ascontiguousarray(np.concatenate([kl, khh], 0))
    m["feats16"] = np.ascontiguousarray(
        np.asarray(inputs["known_feats"], np.float32)[b].T.astype(np.float16))
    m["ufeats16"] = np.ascontiguousarray(
        np.asarray(inputs["unknow_feats"], np.float32)[b, :, h * P:(h + 1) * P].astype(np.float16))
    return m


def _run(inputs, trace=False):
    if "nc" not in _cache:
        _cache["nc"] = _build()
    nc = _cache["nc"]
    shared = _prep_shared(inputs)
    in_maps = [_prep_core(inputs, shared, c // 2, c % 2) for c in range(8)]
    kwargs = {}
    if trace:
        kwargs = dict(trace=True, trace_cores=[0])
    res = run_bass_kernel_spmd(nc, in_maps, core_ids=list(range(8)), **kwargs)
    out = np.zeros((B, C2, N), np.float32)
    for c in range(8):
        out[c // 2, :, (c % 2) * P:(c % 2 + 1) * P] = res.results[c]["out"]
    return out, res


def kernel(**inputs):
    out, _ = _run(inputs, trace=bool(int(os.environ.get("BASSKNN_TRACE", "0"))))
    return out


# revision 15
# speedup vs baseline: 1.7447x; 1.7447x over previous
"""PointNet++ FP module (3-NN interpolate + SharedMLP) on 8 Trainium2 cores.

Sharding: B=4 batches x 2 halves of N=8192 -> 8 cores, 4096 points each.
Per core: -d2 via split-fp16 matmuls (hi*hi K=5 + cross K=10, exact to ~5e-6)
-> ACT fp32 copy -> DVE top-8 (max/max_index, exact selection) -> per-slot
indirect-DMA feature row gathers -> weighted sum + transpose fused into PE
matmuls against diag(w_k) -> fp16 MLP with folded BN + ReLU.
"""

import os
import numpy as np

import concourse.bass as bass
import concourse.tile as tile
from concourse import bacc, mybir
from concourse.bass import IndirectOffsetOnAxis
from concourse.bass_utils import run_bass_kernel_spmd

F32 = mybir.dt.float32
F16 = mybir.dt.float16
U32 = mybir.dt.uint32
AX = mybir.AxisListType
OP = mybir.AluOpType
ACTF = mybir.ActivationFunctionType

B, N, M = 4, 8192, 2048
C1, C2 = 128, 256
P = N // 2          # points per core
NT = P // 128       # 32 n-tiles per core
NG = NT // 4        # 8 groups of 4 tiles (512 points)
BN_EPS = 1e-5

_cache = {}


def _build(dbg=False):
    nc = bacc.Bacc("TRN2", target_bir_lowering=False, debug=False, num_devices=8)

    d_uh = nc.dram_tensor("uh", [5, P], F16, kind="ExternalInput").ap()
    d_ux = nc.dram_tensor("ux", [10, P], F16, kind="ExternalInput").ap()
    d_kh = nc.dram_tensor("kh", [5, M], F16, kind="ExternalInput").ap()
    d_kx = nc.dram_tensor("kx", [10, M], F16, kind="ExternalInput").ap()
    d_feats = nc.dram_tensor("feats16", [M, C2], F16, kind="ExternalInput").ap()
    d_ufeats = nc.dram_tensor("ufeats16", [C1, P], F16, kind="ExternalInput").ap()
    d_w0 = [nc.dram_tensor(f"w0{i}", [128, 256], F16, kind="ExternalInput").ap() for i in range(3)]
    d_w1 = [nc.dram_tensor(f"w1{i}", [128, 256], F16, kind="ExternalInput").ap() for i in range(2)]
    d_b0 = nc.dram_tensor("b0sb", [128, 2], F32, kind="ExternalInput").ap()
    d_b1 = nc.dram_tensor("b1sb", [128, 2], F32, kind="ExternalInput").ap()
    d_ident = nc.dram_tensor("ident", [128, 128], F16, kind="ExternalInput").ap()
    d_epsb = nc.dram_tensor("epsb", [128, 1], F32, kind="ExternalInput").ap()
    d_out = nc.dram_tensor("out", [C2, P], F32, kind="ExternalOutput").ap()
    d_dbg = {}
    if dbg:
        for nm, shp, dt in [
            ("dbg_top8", [128, 8], F32), ("dbg_idx8", [128, 32], U32),
            ("dbg_wt", [128, 12], F32), ("dbg_gf", [128, 3072], F16),
            ("dbg_xa", [128, 512], F16), ("dbg_h1a", [128, 512], F16),
        ]:
            d_dbg[nm] = nc.dram_tensor(nm, shp, dt, kind="ExternalOutput").ap()

    with tile.TileContext(nc) as tc:
        with (
            tc.tile_pool(name="const", bufs=1) as cpool,
            tc.tile_pool(name="neg", bufs=3) as negpool,
            tc.tile_pool(name="sel", bufs=2) as selpool,
            tc.tile_pool(name="mlp", bufs=2) as mlppool,
            tc.tile_pool(name="psum_neg", bufs=2, space="PSUM") as pneg,
            tc.tile_pool(name="psum_mlp", bufs=4, space="PSUM") as pmlp,
        ):
            # ---- persistent constants ----
            uh = cpool.tile([5, P], F16)
            nc.sync.dma_start(uh[:], d_uh)
            ux = cpool.tile([10, P], F16)
            nc.sync.dma_start(ux[:], d_ux)
            kh = cpool.tile([5, M], F16)
            nc.sync.dma_start(kh[:], d_kh)
            kx = cpool.tile([10, M], F16)
            nc.sync.dma_start(kx[:], d_kx)
            w0 = []
            for i in range(3):
                w0t = cpool.tile([128, 256], F16, name=f"w0t{i}")
                nc.sync.dma_start(w0t[:], d_w0[i])
                w0.append(w0t)
            w1 = []
            for i in range(2):
                w1t = cpool.tile([128, 256], F16, name=f"w1t{i}")
                nc.sync.dma_start(w1t[:], d_w1[i])
                w1.append(w1t)
            b0 = cpool.tile([128, 2], F32)
            nc.sync.dma_start(b0[:], d_b0)
            b1 = cpool.tile([128, 2], F32)
            nc.sync.dma_start(b1[:], d_b1)
            ident = cpool.tile([128, 128], F16)
            nc.sync.dma_start(ident[:], d_ident)
            epsb = cpool.tile([128, 1], F32)
            nc.sync.dma_start(epsb[:], d_epsb)

            for g in range(NG):
                # ---- selection: 4 tiles of 128 points ----
                idx8 = selpool.tile([128, 4, 8], U32, name=f"idx8_{g}")
                top8 = selpool.tile([128, 4, 8], F32, name=f"top8_{g}", tag="top8")
                for t in range(4):
                    i = g * 4 + t
                    negsb = negpool.tile([128, M], F32, name=f"negsb_{i}", tag="negsb")
                    for h in range(2):
                        pn = pneg.tile([128, 1024], F32, name=f"pn_{i}_{h}", tag="pn")
                        for c in range(2):
                            sl = slice(h * 1024 + c * 512, h * 1024 + (c + 1) * 512)
                            nc.tensor.matmul(
                                pn[:, c * 512:(c + 1) * 512],
                                uh[:, i * 128:(i + 1) * 128], kh[:, sl],
                                start=True, stop=False,
                            )
                            nc.tensor.matmul(
                                pn[:, c * 512:(c + 1) * 512],
                                ux[:, i * 128:(i + 1) * 128], kx[:, sl],
                                start=False, stop=True,
                            )
                        nc.scalar.activation(
                            negsb[:, h * 1024:(h + 1) * 1024], pn[:], ACTF.Copy
                        )
                    nc.vector.max(top8[:, t, :], negsb[:])
                    nc.vector.max_index(idx8[:, t, :], top8[:, t, :], negsb[:])
                if dbg and g == 0:
                    nc.sync.dma_start(d_dbg["dbg_top8"], top8[:, 0, :])
                    nc.sync.dma_start(d_dbg["dbg_idx8"], idx8[:].rearrange("p t k -> p (t k)"))

                # ---- weights: w_k = (1/(d_k+1e-8)) / sum ----
                dist = selpool.tile([128, 4, 3], F32, name=f"dist_{g}", tag="dist")
                nc.scalar.activation(dist[:], top8[:, :, 0:3], ACTF.Sqrt, bias=epsb[:], scale=-1.0)
                nc.vector.tensor_scalar_add(dist[:], dist[:], 1e-8)
                rec = selpool.tile([128, 4, 3], F32, name=f"rec_{g}", tag="rec")
                nc.vector.reciprocal(rec[:], dist[:])
                rs = selpool.tile([128, 4, 1], F32, name=f"rs_{g}", tag="rs")
                nc.vector.tensor_reduce(rs[:], rec[:], axis=AX.X, op=OP.add)
                nc.vector.reciprocal(rs[:], rs[:])
                wt = selpool.tile([128, 4, 3], F32, name=f"wt_{g}", tag="wt")
                nc.vector.tensor_mul(wt[:], rec[:], rs[:].broadcast_to((128, 4, 3)))
                if dbg and g == 0:
                    nc.sync.dma_start(d_dbg["dbg_wt"], wt[:].rearrange("p t k -> p (t k)"))

                # ---- gather feature rows (single-offset per call) ----
                gf = selpool.tile([128, 12, C2], F16, name=f"gf_{g}", tag="gf")
                for t in range(4):
                    for k in range(3):
                        nc.gpsimd.indirect_dma_start(
                            gf[:, t * 3 + k, :], None, d_feats,
                            IndirectOffsetOnAxis(ap=idx8[:, t, k:k + 1], axis=0),
                        )
                if dbg and g == 0:
                    nc.sync.dma_start(d_dbg["dbg_gf"], gf[:].rearrange("p t c -> p (t c)"))

                # ---- weighted sum on DVE, then transpose via PE ----
                xa = mlppool.tile([128, 512], F16, name=f"xa_{g}", tag="xa")
                xb = mlppool.tile([128, 512], F16, name=f"xb_{g}", tag="xb")
                gs = selpool.tile([128, 4, C2], F16, name=f"gs_{g}", tag="gs")
                for t in range(4):
                    nc.vector.tensor_scalar_mul(gs[:, t, :], gf[:, t * 3, :], wt[:, t, 0:1])
                    for k in (1, 2):
                        nc.vector.scalar_tensor_tensor(
                            gs[:, t, :], gf[:, t * 3 + k, :], wt[:, t, k:k + 1],
                            gs[:, t, :], op0=OP.mult, op1=OP.add)
                for t in range(4):
                    for o, dst in ((0, xa), (1, xb)):
                        pmx = pmlp.tile([128, 128], F32, name=f"pmx_{g}_{t}_{o}", tag="mlp")
                        nc.tensor.matmul(
                            pmx[:], gs[:, t, o * 128:(o + 1) * 128], ident[:],
                            start=True, stop=True)
                        if o == 0:
                            nc.vector.tensor_copy(dst[:, t * 128:(t + 1) * 128], pmx[:])
                        else:
                            nc.scalar.activation(dst[:, t * 128:(t + 1) * 128], pmx[:], ACTF.Copy)
                xc = mlppool.tile([128, 512], F16, name=f"xc_{g}", tag="xc")
                nc.sync.dma_start(xc[:], d_ufeats[:, g * 512:(g + 1) * 512])
                if dbg and g == 0:
                    nc.sync.dma_start(d_dbg["dbg_xa"], xa[:])

                # ---- MLP ----
                xs = [xa, xb, xc]
                h1 = []
                for o in range(2):
                    pm = pmlp.tile([128, 512], F32, name=f"pm1_{g}_{o}", tag="mlp")
                    for kb in range(3):
                        nc.tensor.matmul(
                            pm[:], w0[kb][:, o * 128:(o + 1) * 128], xs[kb][:],
                            start=(kb == 0), stop=(kb == 2),
                        )
                    ho = mlppool.tile([128, 512], F16, name=f"h1_{g}_{o}", tag=f"h1{o}")
                    nc.scalar.activation(ho[:], pm[:], ACTF.Relu, bias=b0[:, o:o + 1])
                    h1.append(ho)
                if dbg and g == 0:
                    nc.sync.dma_start(d_dbg["dbg_h1a"], h1[0][:])
                for o in range(2):
                    pm2 = pmlp.tile([128, 512], F32, name=f"pm2_{g}_{o}", tag="mlp")
                    for kb in range(2):
                        nc.tensor.matmul(
                            pm2[:], w1[kb][:, o * 128:(o + 1) * 128], h1[kb][:],
                            start=(kb == 0), stop=(kb == 1),
                        )
                    ost = mlppool.tile([128, 512], F32, name=f"ost_{g}_{o}", tag=f"ost{o}")
                    nc.vector.tensor_scalar(
                        ost[:], pm2[:], b1[:, o:o + 1], 0.0, op0=OP.add, op1=OP.max
                    )
                    nc.sync.dma_start(d_out[o * 128:(o + 1) * 128, g * 512:(g + 1) * 512], ost[:])

    nc.compile()
    return nc


def _split16(a):
    hi = a.astype(np.float16)
    lo = (a - hi.astype(np.float32)).astype(np.float16)
    return hi, lo


def _prep_shared(inputs):
    sh = {}
    for li, (cin, cout) in enumerate([(C1 + C2, 256), (256, 256)]):
        W = np.asarray(inputs[f"W{li}"], np.float32)
        bb = np.asarray(inputs[f"b{li}"], np.float32)
        gg = np.asarray(inputs[f"g{li}"], np.float32)
        beta = np.asarray(inputs[f"beta{li}"], np.float32)
        rm = np.asarray(inputs[f"rm{li}"], np.float32)
        rv = np.asarray(inputs[f"rv{li}"], np.float32)
        scale = gg / np.sqrt(rv + BN_EPS)
        Wf = (W * scale[:, None]).astype(np.float16)
        bf = ((bb - rm) * scale + beta).astype(np.float32)
        wT = np.ascontiguousarray(Wf.T)                      # (cin, cout)
        for i in range(cin // 128):
            sh[f"w{li}{i}"] = np.ascontiguousarray(wT[i * 128:(i + 1) * 128])
        sh[f"b{li}sb"] = np.ascontiguousarray(bf.reshape(2, 128).T)
    sh["ident"] = np.eye(128, dtype=np.float16)
    sh["epsb"] = np.full((128, 1), 1e-16, np.float32)
    return sh


def _prep_core(inputs, shared, b, h):
    u = np.asarray(inputs["unknown"], np.float32)[b, h * P:(h + 1) * P]   # (P,3)
    k = np.asarray(inputs["known"], np.float32)[b]                        # (M,3)
    un2 = (u * u).sum(1)
    kn2 = (k * k).sum(1)
    uT5 = np.stack([2 * u[:, 0], 2 * u[:, 1], 2 * u[:, 2], un2, np.ones(P, np.float32)])
    kT5 = np.stack([k[:, 0], k[:, 1], k[:, 2], -np.ones(M, np.float32), -kn2])
    uh, ul = _split16(uT5)
    khh, kl = _split16(kT5)
    m = dict(shared)
    m["uh"] = np.ascontiguousarray(uh)
    m["kh"] = np.ascontiguousarray(khh)
    m["ux"] = np.ascontiguousarray(np.concatenate([uh, ul], 0))
    m["kx"] = np.ascontiguousarray(np.concatenate([kl, khh], 0))
    m["feats16"] = np.ascontiguousarray(
        np.asarray(inputs["known_feats"], np.float32)[b].T.astype(np.float16))
    m["ufeats16"] = np.ascontiguousarray(
        np.asarray(inputs["unknow_feats"], np.float32)[b, :, h * P:(h + 1) * P].astype(np.float16))
    return m


def _run(inputs, trace=False):
    if "nc" not in _cache:
        _cache["nc"] = _build()
    nc = _cache["nc"]
    shared = _prep_shared(inputs)
    in_maps = [_prep_core(inputs, shared, c // 2, c % 2) for c in range(8)]
    kwargs = {}
    if trace:
        kwargs = dict(trace=True, trace_cores=[0])
    res = run_bass_kernel_spmd(nc, in_maps, core_ids=list(range(8)), **kwargs)
    out = np.zeros((B, C2, N), np.float32)
    for c in range(8):
        out[c // 2, :, (c % 2) * P:(c % 2 + 1) * P] = res.results[c]["out"]
    return out, res


def kernel(**inputs):
    out, _ = _run(inputs, trace=bool(int(os.environ.get("BASSKNN_TRACE", "0"))))
    return out


# revision 16
# speedup vs baseline: 1.8539x; 1.0626x over previous
"""PointNet++ FP module (3-NN interpolate + SharedMLP) on 8 Trainium2 cores.

Sharding: B=4 batches x 2 halves of N=8192 -> 8 cores, 4096 points each.
Per core: -d2 via split-fp16 matmuls (hi*hi K=5 + cross K=10, exact to ~5e-6)
-> ACT fp32 copy -> DVE top-8 (max/max_index, exact selection) -> per-slot
indirect-DMA feature row gathers -> weighted sum + transpose fused into PE
matmuls against diag(w_k) -> fp16 MLP with folded BN + ReLU.
"""

import os
import numpy as np

import concourse.bass as bass
import concourse.tile as tile
from concourse import bacc, mybir
from concourse.bass import IndirectOffsetOnAxis
from concourse.bass_utils import run_bass_kernel_spmd

F32 = mybir.dt.float32
F16 = mybir.dt.float16
U32 = mybir.dt.uint32
AX = mybir.AxisListType
OP = mybir.AluOpType
ACTF = mybir.ActivationFunctionType

B, N, M = 4, 8192, 2048
C1, C2 = 128, 256
P = N // 2          # points per core
NT = P // 128       # 32 n-tiles per core
NG = NT // 4        # 8 groups of 4 tiles (512 points)
BN_EPS = 1e-5

_cache = {}


def _build(dbg=False):
    nc = bacc.Bacc("TRN2", target_bir_lowering=False, debug=False, num_devices=8)

    d_uh = nc.dram_tensor("uh", [5, P], F16, kind="ExternalInput").ap()
    d_ux = nc.dram_tensor("ux", [10, P], F16, kind="ExternalInput").ap()
    d_kh = nc.dram_tensor("kh", [5, M], F16, kind="ExternalInput").ap()
    d_kx = nc.dram_tensor("kx", [10, M], F16, kind="ExternalInput").ap()
    d_feats = nc.dram_tensor("feats16", [M, C2], F16, kind="ExternalInput").ap()
    d_ufeats = nc.dram_tensor("ufeats16", [C1, P], F16, kind="ExternalInput").ap()
    d_w0 = [nc.dram_tensor(f"w0{i}", [128, 256], F16, kind="ExternalInput").ap() for i in range(3)]
    d_w1 = [nc.dram_tensor(f"w1{i}", [128, 256], F16, kind="ExternalInput").ap() for i in range(2)]
    d_b0 = nc.dram_tensor("b0sb", [128, 2], F32, kind="ExternalInput").ap()
    d_b1 = nc.dram_tensor("b1sb", [128, 2], F32, kind="ExternalInput").ap()
    d_ident = nc.dram_tensor("ident", [128, 128], F16, kind="ExternalInput").ap()
    d_epsb = nc.dram_tensor("epsb", [128, 1], F32, kind="ExternalInput").ap()
    d_out = nc.dram_tensor("out", [C2, P], F32, kind="ExternalOutput").ap()
    d_dbg = {}
    if dbg:
        for nm, shp, dt in [
            ("dbg_top8", [128, 8], F32), ("dbg_idx8", [128, 32], U32),
            ("dbg_wt", [128, 12], F32), ("dbg_gf", [128, 3072], F16),
            ("dbg_xa", [128, 512], F16), ("dbg_h1a", [128, 512], F16),
        ]:
            d_dbg[nm] = nc.dram_tensor(nm, shp, dt, kind="ExternalOutput").ap()

    with tile.TileContext(nc) as tc:
        with (
            tc.tile_pool(name="const", bufs=1) as cpool,
            tc.tile_pool(name="neg", bufs=3) as negpool,
            tc.tile_pool(name="sel", bufs=2) as selpool,
            tc.tile_pool(name="mlp", bufs=2) as mlppool,
            tc.tile_pool(name="psum_neg", bufs=2, space="PSUM") as pneg,
            tc.tile_pool(name="psum_mlp", bufs=4, space="PSUM") as pmlp,
        ):
            # ---- persistent constants ----
            uh = cpool.tile([5, P], F16)
            nc.sync.dma_start(uh[:], d_uh)
            ux = cpool.tile([10, P], F16)
            nc.sync.dma_start(ux[:], d_ux)
            kh = cpool.tile([5, M], F16)
            nc.sync.dma_start(kh[:], d_kh)
            kx = cpool.tile([10, M], F16)
            nc.sync.dma_start(kx[:], d_kx)
            w0 = []
            for i in range(3):
                w0t = cpool.tile([128, 256], F16, name=f"w0t{i}")
                nc.sync.dma_start(w0t[:], d_w0[i])
                w0.append(w0t)
            w1 = []
            for i in range(2):
                w1t = cpool.tile([128, 256], F16, name=f"w1t{i}")
                nc.sync.dma_start(w1t[:], d_w1[i])
                w1.append(w1t)
            b0 = cpool.tile([128, 2], F32)
            nc.sync.dma_start(b0[:], d_b0)
            b1 = cpool.tile([128, 2], F32)
            nc.sync.dma_start(b1[:], d_b1)
            ident = cpool.tile([128, 128], F16)
            nc.sync.dma_start(ident[:], d_ident)
            epsb = cpool.tile([128, 1], F32)
            nc.sync.dma_start(epsb[:], d_epsb)

            for g in range(NG):
                # ---- selection: 4 tiles of 128 points ----
                idx8 = selpool.tile([128, 4, 8], U32, name=f"idx8_{g}")
                top8 = selpool.tile([128, 4, 8], F32, name=f"top8_{g}", tag="top8")
                for t in range(4):
                    i = g * 4 + t
                    negsb = negpool.tile([128, M], F32, name=f"negsb_{i}", tag="negsb")
                    for h in range(2):
                        pn = pneg.tile([128, 1024], F32, name=f"pn_{i}_{h}", tag="pn")
                        for c in range(2):
                            sl = slice(h * 1024 + c * 512, h * 1024 + (c + 1) * 512)
                            nc.tensor.matmul(
                                pn[:, c * 512:(c + 1) * 512],
                                uh[:, i * 128:(i + 1) * 128], kh[:, sl],
                                start=True, stop=False,
                            )
                            nc.tensor.matmul(
                                pn[:, c * 512:(c + 1) * 512],
                                ux[:, i * 128:(i + 1) * 128], kx[:, sl],
                                start=False, stop=True,
                            )
                        nc.scalar.activation(
                            negsb[:, h * 1024:(h + 1) * 1024], pn[:], ACTF.Copy
                        )
                    nc.vector.max(top8[:, t, :], negsb[:])
                    nc.vector.max_index(idx8[:, t, :], top8[:, t, :], negsb[:])
                if dbg and g == 0:
                    nc.sync.dma_start(d_dbg["dbg_top8"], top8[:, 0, :])
                    nc.sync.dma_start(d_dbg["dbg_idx8"], idx8[:].rearrange("p t k -> p (t k)"))

                # ---- weights: w_k = (1/(d_k+1e-8)) / sum ----
                dist = selpool.tile([128, 4, 3], F32, name=f"dist_{g}", tag="dist")
                nc.scalar.activation(dist[:], top8[:, :, 0:3], ACTF.Sqrt, bias=epsb[:], scale=-1.0)
                nc.vector.tensor_scalar_add(dist[:], dist[:], 1e-8)
                rec = selpool.tile([128, 4, 3], F32, name=f"rec_{g}", tag="rec")
                nc.vector.reciprocal(rec[:], dist[:])
                rs = selpool.tile([128, 4, 1], F32, name=f"rs_{g}", tag="rs")
                nc.vector.tensor_reduce(rs[:], rec[:], axis=AX.X, op=OP.add)
                nc.vector.reciprocal(rs[:], rs[:])
                wt = selpool.tile([128, 4, 3], F32, name=f"wt_{g}", tag="wt")
                nc.vector.tensor_mul(wt[:], rec[:], rs[:].broadcast_to((128, 4, 3)))
                if dbg and g == 0:
                    nc.sync.dma_start(d_dbg["dbg_wt"], wt[:].rearrange("p t k -> p (t k)"))

                # ---- gather feature rows (single-offset per call) ----
                gf = selpool.tile([128, 12, C2], F16, name=f"gf_{g}", tag="gf")
                for t in range(4):
                    for k in range(3):
                        nc.gpsimd.indirect_dma_start(
                            gf[:, t * 3 + k, :], None, d_feats,
                            IndirectOffsetOnAxis(ap=idx8[:, t, k:k + 1], axis=0),
                        )
                if dbg and g == 0:
                    nc.sync.dma_start(d_dbg["dbg_gf"], gf[:].rearrange("p t c -> p (t c)"))

                # ---- weighted sum on DVE, then transpose via PE ----
                xa = mlppool.tile([128, 512], F16, name=f"xa_{g}", tag="xa")
                xb = mlppool.tile([128, 512], F16, name=f"xb_{g}", tag="xb")
                gs = selpool.tile([128, 4, C2], F16, name=f"gs_{g}", tag="gs")
                for t in range(4):
                    nc.vector.tensor_scalar_mul(gs[:, t, :], gf[:, t * 3, :], wt[:, t, 0:1])
                    for k in (1, 2):
                        nc.vector.scalar_tensor_tensor(
                            gs[:, t, :], gf[:, t * 3 + k, :], wt[:, t, k:k + 1],
                            gs[:, t, :], op0=OP.mult, op1=OP.add)
                for t in range(4):
                    for o, dst in ((0, xa), (1, xb)):
                        pmx = pmlp.tile([128, 128], F32, name=f"pmx_{g}_{t}_{o}", tag="mlp")
                        nc.tensor.matmul(
                            pmx[:], gs[:, t, o * 128:(o + 1) * 128], ident[:],
                            start=True, stop=True)
                        nc.scalar.activation(dst[:, t * 128:(t + 1) * 128], pmx[:], ACTF.Copy)
                xc = mlppool.tile([128, 512], F16, name=f"xc_{g}", tag="xc")
                nc.sync.dma_start(xc[:], d_ufeats[:, g * 512:(g + 1) * 512])
                if dbg and g == 0:
                    nc.sync.dma_start(d_dbg["dbg_xa"], xa[:])

                # ---- MLP ----
                xs = [xa, xb, xc]
                h1 = []
                for o in range(2):
                    pm = pmlp.tile([128, 512], F32, name=f"pm1_{g}_{o}", tag="mlp")
                    for kb in range(3):
                        nc.tensor.matmul(
                            pm[:], w0[kb][:, o * 128:(o + 1) * 128], xs[kb][:],
                            start=(kb == 0), stop=(kb == 2),
                        )
                    ho = mlppool.tile([128, 512], F16, name=f"h1_{g}_{o}", tag=f"h1{o}")
                    nc.scalar.activation(ho[:], pm[:], ACTF.Relu, bias=b0[:, o:o + 1])
                    h1.append(ho)
                if dbg and g == 0:
                    nc.sync.dma_start(d_dbg["dbg_h1a"], h1[0][:])
                for o in range(2):
                    pm2 = pmlp.tile([128, 512], F32, name=f"pm2_{g}_{o}", tag="mlp")
                    for kb in range(2):
                        nc.tensor.matmul(
                            pm2[:], w1[kb][:, o * 128:(o + 1) * 128], h1[kb][:],
                            start=(kb == 0), stop=(kb == 1),
                        )
                    ost = mlppool.tile([128, 512], F32, name=f"ost_{g}_{o}", tag=f"ost{o}")
                    nc.scalar.activation(ost[:], pm2[:], ACTF.Relu, bias=b1[:, o:o + 1])
                    nc.sync.dma_start(d_out[o * 128:(o + 1) * 128, g * 512:(g + 1) * 512], ost[:])

    nc.compile()
    return nc


def _split16(a):
    hi = a.astype(np.float16)
    lo = (a - hi.astype(np.float32)).astype(np.float16)
    return hi, lo


def _prep_shared(inputs):
    sh = {}
    for li, (cin, cout) in enumerate([(C1 + C2, 256), (256, 256)]):
        W = np.asarray(inputs[f"W{li}"], np.float32)
        bb = np.asarray(inputs[f"b{li}"], np.float32)
        gg = np.asarray(inputs[f"g{li}"], np.float32)
        beta = np.asarray(inputs[f"beta{li}"], np.float32)
        rm = np.asarray(inputs[f"rm{li}"], np.float32)
        rv = np.asarray(inputs[f"rv{li}"], np.float32)
        scale = gg / np.sqrt(rv + BN_EPS)
        Wf = (W * scale[:, None]).astype(np.float16)
        bf = ((bb - rm) * scale + beta).astype(np.float32)
        wT = np.ascontiguousarray(Wf.T)                      # (cin, cout)
        for i in range(cin // 128):
            sh[f"w{li}{i}"] = np.ascontiguousarray(wT[i * 128:(i + 1) * 128])
        sh[f"b{li}sb"] = np.ascontiguousarray(bf.reshape(2, 128).T)
    sh["ident"] = np.eye(128, dtype=np.float16)
    sh["epsb"] = np.full((128, 1), 1e-16, np.float32)
    return sh


def _prep_core(inputs, shared, b, h):
    u = np.asarray(inputs["unknown"], np.float32)[b, h * P:(h + 1) * P]   # (P,3)
    k = np.asarray(inputs["known"], np.float32)[b]                        # (M,3)
    un2 = (u * u).sum(1)
    kn2 = (k * k).sum(1)
    uT5 = np.stack([2 * u[:, 0], 2 * u[:, 1], 2 * u[:, 2], un2, np.ones(P, np.float32)])
    kT5 = np.stack([k[:, 0], k[:, 1], k[:, 2], -np.ones(M, np.float32), -kn2])
    uh, ul = _split16(uT5)
    khh, kl = _split16(kT5)
    m = dict(shared)
    m["uh"] = np.ascontiguousarray(uh)
    m["kh"] = np.ascontiguousarray(khh)
    m["ux"] = np.ascontiguousarray(np.concatenate([uh, ul], 0))
    m["kx"] = np.ascontiguousarray(np.concatenate([kl, khh], 0))
    m["feats16"] = np.ascontiguousarray(
        np.asarray(inputs["known_feats"], np.float32)[b].T.astype(np.float16))
    m["ufeats16"] = np.ascontiguousarray(
        np.asarray(inputs["unknow_feats"], np.float32)[b, :, h * P:(h + 1) * P].astype(np.float16))
    return m


def _run(inputs, trace=False):
    if "nc" not in _cache:
        _cache["nc"] = _build()
    nc = _cache["nc"]
    shared = _prep_shared(inputs)
    in_maps = [_prep_core(inputs, shared, c // 2, c % 2) for c in range(8)]
    kwargs = {}
    if trace:
        kwargs = dict(trace=True, trace_cores=[0])
    res = run_bass_kernel_spmd(nc, in_maps, core_ids=list(range(8)), **kwargs)
    out = np.zeros((B, C2, N), np.float32)
    for c in range(8):
        out[c // 2, :, (c % 2) * P:(c % 2 + 1) * P] = res.results[c]["out"]
    return out, res


def kernel(**inputs):
    out, _ = _run(inputs, trace=bool(int(os.environ.get("BASSKNN_TRACE", "0"))))
    return out
